# revision 17
# baseline (speedup 1.0000x reference)
"""PointSetTracker Trainium2 kernel.

Layout strategy: channels-first activations (C on partitions, K=512 points on
free dim). Attention computed in S^T layout (keys on partitions, queries on
free) so softmax sums become ones-matmuls and no transposes are needed.
LSTM stage runs points-major. Batch (64) is sharded 8-per-core across the 8
NeuronCores; all weights replicated.
"""
import sys
import numpy as np
import ml_dtypes

sys.path.insert(0, "/opt/trn_rl_repo")

import concourse.bass as bass
import concourse.tile as tile
from concourse import bacc, mybir
from concourse.bass_utils import run_bass_kernel_spmd

BF16 = mybir.dt.float16
F32 = mybir.dt.float32
AF = mybir.ActivationFunctionType
ALU = mybir.AluOpType
bf16 = np.float16

B, K, C, H = 64, 512, 128, 4
NL = 8
DIL = (1, 1, 1, 1, 2, 2, 4, 4)
SD, FUS, RNN = 128, 256, 256
NB = 8          # batch elems per core
NCORES = 8
EPS = 1e-6

_CACHE = {}


def _build():
    nc = bacc.Bacc("TRN2", target_bir_lowering=False, debug=False)
    P = 128

    def din(name, shape, dt=BF16):
        return nc.dram_tensor(name, list(shape), dt, kind="ExternalInput").ap()

    # per-core data
    xb = din("xb", (NB, C, K))                       # x slice, channels-first, bf16
    h0b = din("h0b", (NB * K, RNN))                  # bf16 (matmul input)
    c0f = din("c0f", (NB * K, RNN), F32)
    # weights (host-prepped, see kernel())
    wq = din("wq", (C, NL, SD))                      # lhsT per layer (in,out), q pre-scaled
    wk = din("wk", (C, NL, SD))
    wv = din("wv", (C, NL, H * C))
    wfold = din("wfold", (C, NL, 4, SD))             # (Ww @ Wa_top) reshaped
    wabot = din("wabot", (C, NL, SD))
    cwt = din("cwt", (C, NL, 9, SD))                 # conv taps, (c_in, layer, tap, c_out)
    lng = din("lng", (C, NL), F32)
    lnb = din("lnb", (C, NL), F32)
    cbt = din("cbt", (C, NL), F32)
    fusw = din("fusw", (C, NL, FUS))                 # fus_w.T tiles
    fusb = din("fusb", (C, 2), F32)
    rsgw = din("rsgw", (C, 2, FUS))                  # rs_w[:, :256].T tiles
    rssw = din("rssw", (C, NL, FUS))                 # rs_w[:, 256:].T tiles
    rsb = din("rsb", (C, 2), F32)
    wih = din("wih", (C, 2, 4 * RNN))                # W_ih.T tiles
    whh = din("whh", (C, 2, 4 * RNN))
    bcomb = din("bcomb", (1, 4 * RNN))               # b_ih + b_hh (bf16)
    p1ow = din("p1ow", (C, 2, 256))
    p1gw = din("p1gw", (C, 2, 256))
    p1sw = din("p1sw", (C, NL, 256))
    p1b = din("p1b", (C, 2), F32)
    p2w = din("p2w", (C, 2, 64))
    p2b = din("p2b", (C, 1), F32)
    p3w = din("p3w", (C, 2))                         # p3_w.T padded to 128 rows
    p3b = din("p3b", (C, 1), F32)
    ident = din("ident", (C, C))                     # identity for PE transpose

    p_out = nc.dram_tensor("p_out", [NB, 2, K], F32, kind="ExternalOutput").ap()
    h_out = nc.dram_tensor("h_out", [NB * K, RNN], F32, kind="ExternalOutput").ap()
    c_out = nc.dram_tensor("c_out", [NB * K, RNN], F32, kind="ExternalOutput").ap()

    with tile.TileContext(nc) as tc:
        with (
            tc.tile_pool(name="wp", bufs=1) as wp,
            tc.tile_pool(name="states", bufs=2) as statesp,
            tc.tile_pool(name="big", bufs=8) as bigp,
            tc.tile_pool(name="work", bufs=2) as work,
            tc.tile_pool(name="work3", bufs=3) as work3,
            tc.tile_pool(name="hp", bufs=2) as hp,
            tc.tile_pool(name="dram", bufs=3, space="DRAM") as dramp,
            tc.tile_pool(name="dram_states", bufs=NB, space="DRAM") as dram_states,
            tc.tile_pool(name="ps_st", bufs=1, space="PSUM") as ps_st,
            tc.tile_pool(name="ps_rt", bufs=2, space="PSUM") as ps_rt,
            tc.tile_pool(name="ps_sums", bufs=1, space="PSUM") as ps_sums,
            tc.tile_pool(name="ps_mm", bufs=3, space="PSUM") as ps_mm,
        ):
            # ---- load weights to SBUF once
            def wload(ap_in, shape, dt=BF16, tag=None):
                t = wp.tile(list(shape), dt, tag=tag or ap_in.tensor.name)
                nc.sync.dma_start(t[:], ap_in)
                return t

            wq_s = wload(wq, (C, NL, SD))
            wk_s = wload(wk, (C, NL, SD))
            wv_s = wload(wv, (C, NL, H * C))
            wfold_s = wload(wfold, (C, NL, 4, SD))
            wabot_s = wload(wabot, (C, NL, SD))
            cwt_s = wload(cwt, (C, NL, 9, SD))
            lng_s = wload(lng, (C, NL), F32)
            lnb_s = wload(lnb, (C, NL), F32)
            cbt_s = wload(cbt, (C, NL), F32)
            fusw_s = wload(fusw, (C, NL, FUS))
            fusb_s = wload(fusb, (C, 2), F32)
            rsgw_s = wload(rsgw, (C, 2, FUS))
            rssw_s = wload(rssw, (C, NL, FUS))
            rsb_s = wload(rsb, (C, 2), F32)
            wih_s = wload(wih, (C, 2, 4 * RNN))
            whh_s = wload(whh, (C, 2, 4 * RNN))
            p1ow_s = wload(p1ow, (C, 2, 256))
            p1gw_s = wload(p1gw, (C, 2, 256))
            p1sw_s = wload(p1sw, (C, NL, 256))
            p1b_s = wload(p1b, (C, 2), F32)
            p2w_s = wload(p2w, (C, 2, 64))
            p2b_s = wload(p2b, (C, 1), F32)
            p3w_s = wload(p3w, (C, 2))
            p3b_s = wload(p3b, (C, 1), F32)
            ident_s = wload(ident, (C, C))
            bcomb_bc = wp.tile([P, 4 * RNN], BF16, tag="bcomb_bc")
            nc.sync.dma_start(bcomb_bc[:], bcomb.to_broadcast((P, 4 * RNN)))
            onem = wp.tile([P, 32], BF16, tag="onem")   # value 1/128 (LN stats)
            nc.vector.memset(onem[:], 1.0 / 128.0)
            ones32 = wp.tile([P, 32], BF16, tag="ones32")
            nc.vector.memset(ones32[:], 1.0)
            eps_s = wp.tile([1, 1], F32, tag="eps_s")
            nc.vector.memset(eps_s[:], EPS)

            st_dram = [dram_states.tile([P, NL, K], BF16, tag="st_dram", name=f"st_dram_{b}") for b in range(NB)]

            # ================= main loop: 8 batches x 8 layers =================
            for b in range(NB):
                x0 = work.tile([P, K], BF16, tag="x0")
                nc.sync.dma_start(x0[:], xb[b])
                x_bf = x0
                for i in range(NL):
                    d = DIL[i]

                    # ---- LN stats (channels = partitions)
                    x_sq = work.tile([P, K], BF16, tag="x_sq")
                    nc.gpsimd.tensor_tensor(x_sq[:], x_bf, x_bf, op=ALU.mult)
                    mean_ps = ps_mm.tile([1, K], F32, tag="mm", name=f"mean_{b}_{i}")
                    nc.tensor.matmul(mean_ps[:], onem[:, 0:1], x_bf, start=True, stop=True)
                    ex2_ps = ps_mm.tile([1, K], F32, tag="mm", name=f"ex2_{b}_{i}")
                    nc.tensor.matmul(ex2_ps[:], onem[:, 0:1], x_sq[:], start=True, stop=True)
                    lnrow = work.tile([1, K], F32, tag="lnrow")
                    nc.scalar.activation(lnrow[:], mean_ps[:], func=AF.Square)
                    nc.vector.tensor_sub(lnrow[:], ex2_ps[:], lnrow[:])
                    nc.scalar.activation(lnrow[:], lnrow[:], func=AF.Ln, bias=eps_s[:])
                    ln_rows = work.tile([1, 2, K], BF16, tag="ln_rows")
                    # rstd = exp(-0.5*ln(var+eps))
                    nc.scalar.activation(ln_rows[:, 0, :], lnrow[:], func=AF.Exp, scale=-0.5)
                    # mu*rstd
                    nc.vector.tensor_mul(ln_rows[:, 1, :], mean_ps[:], ln_rows[:, 0, :])
                    ln_dram = dramp.tile([1, 2 * K], BF16, tag="ln_dram")
                    nc.sync.dma_start(ln_dram[:], ln_rows[:].rearrange("p a k -> p (a k)"))
                    ln_bc = work.tile([P, 2, K], BF16, tag="ln_bc")
                    nc.sync.dma_start(
                        ln_bc[:].rearrange("p a k -> p (a k)"),
                        ln_dram[:].to_broadcast((P, 2 * K)),
                    )
                    # ln = (x*rstd - mu*rstd)*g + b
                    t1 = work.tile([P, K], BF16, tag="t1")
                    nc.vector.tensor_mul(t1[:], x_bf, ln_bc[:, 0, :])
                    t2 = work.tile([P, K], BF16, tag="t2")
                    nc.vector.tensor_sub(t2[:], t1[:], ln_bc[:, 1, :])
                    ln_cf = work.tile([P, K], BF16, tag="ln_cf")
                    nc.vector.tensor_scalar(
                        ln_cf[:], t2[:], lng_s[:, i : i + 1], lnb_s[:, i : i + 1],
                        op0=ALU.mult, op1=ALU.add,
                    )

                    # ---- q^T, k^T  (feature-major, (128, 512))
                    qT_ps = ps_mm.tile([P, K], F32, tag="mm", name=f"q_{b}_{i}")
                    nc.tensor.matmul(qT_ps[:], wq_s[:, i, :], ln_cf[:], start=True, stop=True)
                    q_sb = work.tile([P, K], BF16, tag="q_sb")
                    nc.vector.tensor_copy(q_sb[:], qT_ps[:])
                    kT_ps = ps_mm.tile([P, K], F32, tag="mm", name=f"k_{b}_{i}")
                    nc.tensor.matmul(kT_ps[:], wk_s[:, i, :], ln_cf[:], start=True, stop=True)
                    k_sb = work.tile([P, K], BF16, tag="k_sb")
                    nc.vector.tensor_copy(k_sb[:], kT_ps[:])

                    # ---- v (points-major rows, (128 pts, 512 feat) per j-tile)
                    v_sb = work.tile([P, 4, H * C], BF16, tag="v_sb")
                    for jt in range(4):
                        v_ps = ps_mm.tile([P, H * C], F32, tag="mm", name=f"v_{b}_{i}_{jt}")
                        nc.tensor.matmul(
                            v_ps[:], ln_cf[:, jt * 128 : (jt + 1) * 128], wv_s[:, i, :],
                            start=True, stop=True,
                        )
                        if jt % 2 == 0:
                            nc.vector.tensor_copy(v_sb[:, jt, :], v_ps[:])
                        else:
                            nc.scalar.copy(v_sb[:, jt, :], v_ps[:])

                    # ---- S^T + exp, per j-tile; heads row-tiled, 2-head PSUM groups
                    at = [[None, None] for _ in range(4)]
                    for jt in range(4):
                        for g in range(2):
                            st_ps = ps_st.tile(
                                [P, 2, K], F32, tag="st", name=f"st_{b}_{i}_{jt}_{g}"
                            )
                            for hh in range(2):
                                h = 2 * g + hh
                                nc.tensor.matmul(
                                    st_ps[:, hh, :],
                                    k_sb[32 * h : 32 * (h + 1), jt * 128 : (jt + 1) * 128],
                                    q_sb[32 * h : 32 * (h + 1), :],
                                    start=True, stop=True, tile_position=(32 * h, 0),
                                )
                            a_t = bigp.tile(
                                [P, 2, K], BF16, tag="at", name=f"at_{b}_{i}_{jt}_{g}"
                            )
                            nc.scalar.activation(a_t[:], st_ps[:], func=AF.Exp)
                            at[jt][g] = a_t

                    # ---- softmax denominators: col-tiled ones-matmuls
                    sums_ps = ps_sums.tile([P, K], F32, tag="sums", name=f"sums_{b}_{i}")
                    for h in range(H):
                        for jt in range(4):
                            nc.tensor.matmul(
                                sums_ps[32 * h : 32 * (h + 1), :],
                                ones32[:], at[jt][h // 2][:, h % 2, :],
                                start=(jt == 0), stop=(jt == 3),
                                tile_position=(0, 32 * h),
                            )
                    rl = work.tile([P, K], F32, tag="rl")
                    nc.scalar.activation(rl[:], sums_ps[:], func=AF.Ln)
                    recip = work.tile([P, K], BF16, tag="recip")
                    nc.scalar.activation(recip[:], rl[:], func=AF.Exp, scale=-1.0)
                    rc_dram = dramp.tile([1, H * K], BF16, tag="rc_dram")
                    for h in range(H):
                        nc.sync.dma_start(
                            rc_dram[:, h * K : (h + 1) * K], recip[32 * h : 32 * h + 1, :]
                        )
                    rc_bc = work.tile([P, H, K], BF16, tag="rc_bc")
                    nc.sync.dma_start(
                        rc_bc[:].rearrange("p h k -> p (h k)"),
                        rc_dram[:].to_broadcast((P, H * K)),
                    )

                    # ---- r^T = v^T @ A^T (unnormalized), then normalize per head
                    r_sb = work.tile([P, H, K], BF16, tag="r_sb")
                    for h in range(H):
                        rt_ps = ps_rt.tile([P, K], F32, tag="rt", name=f"rt_{b}_{i}_{h}")
                        for jt in range(4):
                            nc.tensor.matmul(
                                rt_ps[:],
                                v_sb[:, jt, h * 128 : (h + 1) * 128],
                                at[jt][h // 2][:, h % 2, :],
                                start=(jt == 0), stop=(jt == 3),
                            )
                        nc.vector.tensor_mul(r_sb[:, h, :], rt_ps[:], rc_bc[:, h, :])

                    # ---- y^T = Wfold.T @ r^T + Wa_bot.T @ x
                    y_ps = ps_mm.tile([P, K], F32, tag="mm", name=f"y_{b}_{i}")
                    for kt in range(4):
                        nc.tensor.matmul(
                            y_ps[:], wfold_s[:, i, kt, :], r_sb[:, kt, :],
                            start=(kt == 0), stop=False,
                        )
                    nc.tensor.matmul(y_ps[:], wabot_s[:, i, :], x_bf, start=False, stop=True)

                    # ---- circular pad + dilated conv (9 taps)
                    y_pad = work.tile([P, K + 32], BF16, tag="y_pad")
                    nc.vector.tensor_copy(y_pad[:, 4 * d : 4 * d + K], y_ps[:])
                    nc.vector.tensor_copy(y_pad[:, 0 : 4 * d], y_pad[:, K : K + 4 * d])
                    nc.vector.tensor_copy(
                        y_pad[:, 4 * d + K : 8 * d + K], y_pad[:, 4 * d : 8 * d]
                    )
                    c_ps = ps_mm.tile([P, K], F32, tag="mm", name=f"c_{b}_{i}")
                    for t in range(9):
                        nc.tensor.matmul(
                            c_ps[:], cwt_s[:, i, t, :], y_pad[:, t * d : t * d + K],
                            start=(t == 0), stop=(t == 8),
                        )
                    # relu(conv + cb) (+ residual)
                    if i == 0:
                        h_new = hp.tile([P, K], F32, tag="h")
                        nc.vector.tensor_scalar(
                            h_new[:], c_ps[:], cbt_s[:, i : i + 1], 0.0,
                            op0=ALU.add, op1=ALU.max,
                        )
                    else:
                        t_relu = work.tile([P, K], F32, tag="t_relu")
                        nc.vector.tensor_scalar(
                            t_relu[:], c_ps[:], cbt_s[:, i : i + 1], 0.0,
                            op0=ALU.add, op1=ALU.max,
                        )
                        h_prev = h_cur
                        h_new = hp.tile([P, K], F32, tag="h")
                        nc.vector.tensor_add(h_new[:], t_relu[:], h_prev[:])
                    h_cur = h_new
                    hbf = work3.tile([P, K], BF16, tag="hbf")
                    nc.vector.tensor_copy(hbf[:], h_cur[:])
                    nc.sync.dma_start(st_dram[b][:, i, :], hbf[:])
                    x_bf = hbf

            # ================= tail: fusion + LSTM + head, per batch =================
            for b in range(NB):
                st_b = statesp.tile([P, NL, K], BF16, tag="st_b")
                nc.sync.dma_start(st_b[:], st_dram[b][:])
                # h0 channels-first via DMA transpose
                h0cf = work.tile([P, 2, K], BF16, tag="h0cf")
                for ct in range(2):
                    nc.sync.dma_start_transpose(
                        h0cf[:, ct, :],
                        h0b[b * K : (b + 1) * K, ct * 128 : (ct + 1) * 128],
                    )

                # fus conv1x1 + max over K -> g
                g_col = work.tile([P, 2, 1], F32, tag="g_col")
                g_bf = work.tile([P, 2, 1], BF16, tag="g_bf")
                for mt in range(2):
                    f_ps = ps_mm.tile([P, K], F32, tag="mm", name=f"f_{b}_{mt}")
                    for kt in range(NL):
                        nc.tensor.matmul(
                            f_ps[:], fusw_s[:, kt, mt * 128 : (mt + 1) * 128],
                            st_b[:, kt, :], start=(kt == 0), stop=(kt == 7),
                        )
                    gm = work.tile([P, 1], F32, tag="gm")
                    nc.vector.tensor_reduce(gm[:], f_ps[:], axis=mybir.AxisListType.X, op=ALU.max)
                    nc.vector.tensor_scalar(
                        g_col[:, mt, :], gm[:], fusb_s[:, mt : mt + 1], None, op0=ALU.add
                    )
                    nc.vector.tensor_copy(g_bf[:, mt, :], g_col[:, mt, :])

                # g contributions to rs and p1 (rank-1, N=1 matmuls) + biases
                rs_bias = work.tile([P, 2, 1], F32, tag="rs_bias")
                p1_bias = work.tile([P, 2, 1], F32, tag="p1_bias")
                for mt in range(2):
                    rg_ps = ps_mm.tile([P, 1], F32, tag="mm", name=f"rg_{b}_{mt}")
                    for ct in range(2):
                        nc.tensor.matmul(
                            rg_ps[:], rsgw_s[:, ct, mt * 128 : (mt + 1) * 128],
                            g_bf[:, ct, :], start=(ct == 0), stop=(ct == 1),
                        )
                    nc.vector.tensor_scalar(
                        rs_bias[:, mt, :], rg_ps[:], rsb_s[:, mt : mt + 1], None, op0=ALU.add
                    )
                    pg_ps = ps_mm.tile([P, 1], F32, tag="mm", name=f"pg_{b}_{mt}")
                    for ct in range(2):
                        nc.tensor.matmul(
                            pg_ps[:], p1gw_s[:, ct, mt * 128 : (mt + 1) * 128],
                            g_bf[:, ct, :], start=(ct == 0), stop=(ct == 1),
                        )
                    nc.vector.tensor_scalar(
                        p1_bias[:, mt, :], pg_ps[:], p1b_s[:, mt : mt + 1], None, op0=ALU.add
                    )

                # rnn_in channels-first
                rnn_cf = work.tile([P, 2, K], BF16, tag="rnn_cf")
                for mt in range(2):
                    rn_ps = ps_mm.tile([P, K], F32, tag="mm", name=f"rn_{b}_{mt}")
                    for kt in range(NL):
                        nc.tensor.matmul(
                            rn_ps[:], rssw_s[:, kt, mt * 128 : (mt + 1) * 128],
                            st_b[:, kt, :], start=(kt == 0), stop=(kt == 7),
                        )
                    nc.vector.tensor_scalar(
                        rnn_cf[:, mt, :], rn_ps[:], rs_bias[:, mt, :], None, op0=ALU.add
                    )

                # LSTM cell, points-major per k-tile
                o_cf = work.tile([P, 2, K], BF16, tag="o_cf")
                for kt in range(4):
                    gacts = []
                    for n2 in range(2):
                        g_ps = ps_mm.tile([P, 512], F32, tag="mm", name=f"g_{b}_{kt}_{n2}")
                        first = True
                        for ct in range(2):
                            nc.tensor.matmul(
                                g_ps[:], rnn_cf[:, ct, kt * 128 : (kt + 1) * 128],
                                wih_s[:, ct, n2 * 512 : (n2 + 1) * 512],
                                start=first, stop=False,
                            )
                            first = False
                        for ct in range(2):
                            nc.tensor.matmul(
                                g_ps[:], h0cf[:, ct, kt * 128 : (kt + 1) * 128],
                                whh_s[:, ct, n2 * 512 : (n2 + 1) * 512],
                                start=False, stop=(ct == 1),
                            )
                        ga = work.tile([P, 512], F32, tag="gact")
                        nc.vector.tensor_add(ga[:], g_ps[:], bcomb_bc[:, n2 * 512 : (n2 + 1) * 512])
                        gacts.append(ga)
                    # nonlinearities: [i,f] sigmoid, [g] tanh, [o] sigmoid
                    nc.scalar.activation(gacts[0][:], gacts[0][:], func=AF.Sigmoid)
                    nc.scalar.activation(gacts[1][:, 0:256], gacts[1][:, 0:256], func=AF.Tanh)
                    nc.scalar.activation(gacts[1][:, 256:512], gacts[1][:, 256:512], func=AF.Sigmoid)

                    c0t = work.tile([P, RNN], F32, tag="c0t")
                    nc.sync.dma_start(c0t[:], c0f[b * K + kt * 128 : b * K + (kt + 1) * 128, :])
                    t_a = work.tile([P, RNN], F32, tag="t_a")
                    nc.vector.tensor_mul(t_a[:], gacts[0][:, 256:512], c0t[:])
                    t_b = work.tile([P, RNN], F32, tag="t_b")
                    nc.gpsimd.tensor_tensor(
                        t_b[:], gacts[0][:, 0:256], gacts[1][:, 0:256], op=ALU.mult
                    )
                    c_new = work.tile([P, RNN], F32, tag="c_new")
                    nc.vector.tensor_add(c_new[:], t_a[:], t_b[:])
                    nc.sync.dma_start(
                        c_out[b * K + kt * 128 : b * K + (kt + 1) * 128, :], c_new[:]
                    )
                    tanh_c = work.tile([P, RNN], F32, tag="tanh_c")
                    nc.scalar.activation(tanh_c[:], c_new[:], func=AF.Tanh)
                    h_new = work.tile([P, RNN], F32, tag="hn")
                    nc.vector.tensor_mul(h_new[:], gacts[1][:, 256:512], tanh_c[:])
                    nc.sync.dma_start(
                        h_out[b * K + kt * 128 : b * K + (kt + 1) * 128, :], h_new[:]
                    )
                    hn_bf = work.tile([P, RNN], BF16, tag="hn_bf")
                    nc.vector.tensor_copy(hn_bf[:], h_new[:])
                    for ct in range(2):
                        tp_ps = ps_mm.tile([P, C], BF16, tag="mm", name=f"tp_{b}_{kt}_{ct}")
                        nc.tensor.transpose(
                            tp_ps[:], hn_bf[:, ct * 128 : (ct + 1) * 128], ident_s[:]
                        )
                        nc.vector.tensor_copy(o_cf[:, ct, kt * 128 : (kt + 1) * 128], tp_ps[:])

                # p1 -> p2 -> p3
                p1_sb = work.tile([P, 2, K], BF16, tag="p1_sb")
                for mt in range(2):
                    q_ps = ps_mm.tile([P, K], F32, tag="mm", name=f"p1_{b}_{mt}")
                    first = True
                    for ct in range(2):
                        nc.tensor.matmul(
                            q_ps[:], p1ow_s[:, ct, mt * 128 : (mt + 1) * 128],
                            o_cf[:, ct, :], start=first, stop=False,
                        )
                        first = False
                    for kt in range(NL):
                        nc.tensor.matmul(
                            q_ps[:], p1sw_s[:, kt, mt * 128 : (mt + 1) * 128],
                            st_b[:, kt, :], start=False, stop=(kt == 7),
                        )
                    nc.vector.tensor_scalar(
                        p1_sb[:, mt, :], q_ps[:], p1_bias[:, mt, :], 0.0,
                        op0=ALU.add, op1=ALU.max,
                    )
                p2_ps = ps_mm.tile([P, K], F32, tag="mm", name=f"p2_{b}")
                for ct in range(2):
                    nc.tensor.matmul(
                        p2_ps[0:64, :], p2w_s[:, ct, :], p1_sb[:, ct, :],
                        start=(ct == 0), stop=(ct == 1),
                    )
                p2_sb = work.tile([64, K], BF16, tag="p2_sb")
                nc.vector.tensor_scalar(
                    p2_sb[:], p2_ps[0:64, :], p2b_s[0:64, :], 0.0, op0=ALU.add, op1=ALU.max
                )
                p3_ps = ps_mm.tile([P, K], F32, tag="mm", name=f"p3_{b}")
                nc.tensor.matmul(p3_ps[0:2, :], p3w_s[0:64, :], p2_sb[:], start=True, stop=True)
                p_sb = work.tile([2, K], F32, tag="p_sb")
                nc.vector.tensor_scalar(
                    p_sb[:], p3_ps[0:2, :], p3b_s[0:2, :], None, op0=ALU.add
                )
                nc.sync.dma_start(p_out[b], p_sb[:])

    nc.compile()
    return nc


def _prep_inputs(inputs):
    """Host-side weight prep (shared across cores)."""
    f32 = np.float32
    Wq = np.asarray(inputs["Wq"], f32)
    Wk = np.asarray(inputs["Wk"], f32)
    Wv = np.asarray(inputs["Wv"], f32)
    Ww = np.asarray(inputs["Ww"], f32)
    Wa = np.asarray(inputs["Wa"], f32)
    cw = np.asarray(inputs["cw"], f32)

    scale = 1.0 / np.sqrt(np.float32(SD // 4))
    wq = np.ascontiguousarray((Wq * scale).transpose(1, 0, 2)).astype(bf16)  # (C, NL, SD)
    wk = np.ascontiguousarray(Wk.transpose(1, 0, 2)).astype(bf16)
    wv = np.ascontiguousarray(Wv.transpose(1, 0, 2)).astype(bf16)
    # fold Ww @ Wa_top : (NL, 512, 128)
    wf = np.einsum("lfk,lko->lfo", Ww.astype(np.float64), Wa[:, :SD, :].astype(np.float64)).astype(f32)
    wfold = np.ascontiguousarray(
        wf.reshape(NL, 4, SD, SD).transpose(2, 0, 1, 3)
    ).astype(bf16)                                                           # (C, NL, 4, SD)
    wabot = np.ascontiguousarray(Wa[:, SD:, :].transpose(1, 0, 2)).astype(bf16)
    cwt = np.ascontiguousarray(cw.transpose(2, 0, 3, 1)).astype(bf16)        # (C, NL, 9, SD)

    fus_w = np.asarray(inputs["fus_w"], f32)       # (256, 1024)
    fusw = np.ascontiguousarray(
        fus_w.T.reshape(NL, 128, FUS).transpose(1, 0, 2)
    ).astype(bf16)                                                           # (C, NL, FUS)
    rs_w = np.asarray(inputs["rs_w"], f32)         # (256, 1280)
    rsgw = np.ascontiguousarray(
        rs_w[:, :FUS].T.reshape(2, 128, FUS).transpose(1, 0, 2)
    ).astype(bf16)
    rssw = np.ascontiguousarray(
        rs_w[:, FUS:].T.reshape(NL, 128, FUS).transpose(1, 0, 2)
    ).astype(bf16)
    W_ih = np.asarray(inputs["W_ih"], f32)         # (1024, 256)
    W_hh = np.asarray(inputs["W_hh"], f32)
    wih = np.ascontiguousarray(W_ih.T.reshape(2, 128, 4 * RNN)).transpose(1, 0, 2)
    wih = np.ascontiguousarray(wih).astype(bf16)
    whh = np.ascontiguousarray(W_hh.T.reshape(2, 128, 4 * RNN).transpose(1, 0, 2)).astype(bf16)
    bcomb = ((np.asarray(inputs["b_ih"], f32) + np.asarray(inputs["b_hh"], f32)).reshape(1, -1)).astype(bf16)

    p1_w = np.asarray(inputs["p1_w"], f32)         # (256, 1536)
    p1ow = np.ascontiguousarray(p1_w[:, :256].T.reshape(2, 128, 256).transpose(1, 0, 2)).astype(bf16)
    p1gw = np.ascontiguousarray(p1_w[:, 256:512].T.reshape(2, 128, 256).transpose(1, 0, 2)).astype(bf16)
    p1sw = np.ascontiguousarray(p1_w[:, 512:].T.reshape(NL, 128, 256).transpose(1, 0, 2)).astype(bf16)
    p2_w = np.asarray(inputs["p2_w"], f32)         # (64, 256)
    p2w = np.ascontiguousarray(p2_w.T.reshape(2, 128, 64)).transpose(1, 0, 2)
    p2w = np.ascontiguousarray(p2w).astype(bf16)
    p3_w = np.asarray(inputs["p3_w"], f32)         # (2, 64)
    p3w = np.zeros((C, 2), f32)
    p3w[:64, :] = p3_w.T
    p3w = p3w.astype(bf16)

    def cols(v, n):
        return np.ascontiguousarray(np.asarray(v, f32).reshape(n, C).T)

    w = dict(
        wq=wq, wk=wk, wv=wv, wfold=wfold, wabot=wabot, cwt=cwt,
        lng=cols(inputs["ln_g"], NL), lnb=cols(inputs["ln_b"], NL),
        cbt=cols(inputs["cb"], NL),
        fusw=fusw, fusb=cols(inputs["fus_b"], 2),
        rsgw=rsgw, rssw=rssw, rsb=cols(inputs["rs_b"], 2),
        wih=wih, whh=whh, bcomb=bcomb,
        p1ow=p1ow, p1gw=p1gw, p1sw=p1sw, p1b=cols(inputs["p1_b"], 2),
        p2w=p2w, p2b=np.concatenate([np.asarray(inputs["p2_b"], f32), np.zeros(64, f32)]).reshape(C, 1),
        p3w=p3w, p3b=np.concatenate([np.asarray(inputs["p3_b"], f32), np.zeros(126, f32)]).reshape(C, 1),
        ident=np.eye(C, dtype=f32).astype(bf16),
    )
    return w


def kernel(**inputs):
    if "nc" not in _CACHE:
        _CACHE["nc"] = _build()
    nc = _CACHE["nc"]

    w = _prep_inputs(inputs)
    x = np.asarray(inputs["x"], np.float32)
    h0 = np.asarray(inputs["h0"], np.float32)[0]
    c0 = np.asarray(inputs["c0"], np.float32)[0]

    in_maps = []
    for c in range(NCORES):
        bs = c * NB
        m = dict(w)
        m["xb"] = np.ascontiguousarray(x[bs : bs + NB]).astype(bf16)
        m["h0b"] = np.ascontiguousarray(h0[bs * K : (bs + NB) * K]).astype(bf16)
        m["c0f"] = np.ascontiguousarray(c0[bs * K : (bs + NB) * K])
        in_maps.append(m)

    res = run_bass_kernel_spmd(nc, in_maps, core_ids=list(range(NCORES)))

    p = np.concatenate([res.results[c]["p_out"] for c in range(NCORES)], axis=0)
    h_new = np.concatenate([res.results[c]["h_out"] for c in range(NCORES)], axis=0)
    c_new = np.concatenate([res.results[c]["c_out"] for c in range(NCORES)], axis=0)
    return (
        p.astype(np.float32),
        h_new[None].astype(np.float32),
        c_new[None].astype(np.float32),
    )


# revision 18
# speedup vs baseline: 1.0950x; 1.0950x over previous
"""PointSetTracker Trainium2 kernel.

Layout strategy: channels-first activations (C on partitions, K=512 points on
free dim). Attention computed in S^T layout (keys on partitions, queries on
free) so softmax sums become ones-matmuls and no transposes are needed.
LSTM stage runs points-major. Batch (64) is sharded 8-per-core across the 8
NeuronCores; all weights replicated.
"""
import sys
import numpy as np
import ml_dtypes

sys.path.insert(0, "/opt/trn_rl_repo")

import concourse.bass as bass
import concourse.tile as tile
from concourse import bacc, mybir
from concourse.bass_utils import run_bass_kernel_spmd

# Steer the greedy ACT-table chooser: make exp/ln/square resolve only in
# natural_log_exp_and_others and tanh only in sigmoid_and_others, so the main
# loop uses one table set and the LSTM tail another (2 loads total instead of
# ~258 alternating reloads). Set IDs (json positions) are preserved.
_orig_get_tables = bacc.get_activation_tables


def _patched_get_tables(arch):
    tables = dict(_orig_get_tables(arch))
    out = {}
    for name, funcs in tables.items():
        funcs = set(funcs)
        if name != "natural_log_exp_and_others":
            funcs -= {
                mybir.ActivationFunctionType.Exp,
                mybir.ActivationFunctionType.Ln,
                mybir.ActivationFunctionType.Square,
            }
        if name not in ("natural_log_exp_and_others", "sigmoid_and_others"):
            funcs -= {mybir.ActivationFunctionType.Tanh}
        out[name] = funcs
    return out


bacc.get_activation_tables = _patched_get_tables

BF16 = mybir.dt.float16
F32 = mybir.dt.float32
AF = mybir.ActivationFunctionType
ALU = mybir.AluOpType
bf16 = np.float16

B, K, C, H = 64, 512, 128, 4
NL = 8
DIL = (1, 1, 1, 1, 2, 2, 4, 4)
SD, FUS, RNN = 128, 256, 256
NB = 8          # batch elems per core
NCORES = 8
EPS = 1e-6

_CACHE = {}


def _build():
    nc = bacc.Bacc("TRN2", target_bir_lowering=False, debug=False)
    P = 128

    def din(name, shape, dt=BF16):
        return nc.dram_tensor(name, list(shape), dt, kind="ExternalInput").ap()

    # per-core data
    xb = din("xb", (NB, C, K))                       # x slice, channels-first, bf16
    h0b = din("h0b", (NB * K, RNN))                  # bf16 (matmul input)
    c0f = din("c0f", (NB * K, RNN), F32)
    # weights (host-prepped, see kernel())
    wq = din("wq", (C, NL, SD))                      # lhsT per layer (in,out), q pre-scaled
    wk = din("wk", (C, NL, SD))
    wv = din("wv", (C, NL, H * C))
    wfold = din("wfold", (C, NL, 4, SD))             # (Ww @ Wa_top) reshaped
    wabot = din("wabot", (C, NL, SD))
    cwt = din("cwt", (C, NL, 9, SD))                 # conv taps, (c_in, layer, tap, c_out)
    lng = din("lng", (C, NL), F32)
    lnb = din("lnb", (C, NL), F32)
    cbt = din("cbt", (C, NL), F32)
    fusw = din("fusw", (C, NL, FUS))                 # fus_w.T tiles
    fusb = din("fusb", (C, 2), F32)
    rsgw = din("rsgw", (C, 2, FUS))                  # rs_w[:, :256].T tiles
    rssw = din("rssw", (C, NL, FUS))                 # rs_w[:, 256:].T tiles
    rsb = din("rsb", (C, 2), F32)
    wih = din("wih", (C, 2, 4 * RNN))                # W_ih.T tiles
    whh = din("whh", (C, 2, 4 * RNN))
    bcomb = din("bcomb", (1, 4 * RNN))               # b_ih + b_hh (bf16)
    p1ow = din("p1ow", (C, 2, 256))
    p1gw = din("p1gw", (C, 2, 256))
    p1sw = din("p1sw", (C, NL, 256))
    p1b = din("p1b", (C, 2), F32)
    p2w = din("p2w", (C, 2, 64))
    p2b = din("p2b", (C, 1), F32)
    p3w = din("p3w", (C, 2))                         # p3_w.T padded to 128 rows
    p3b = din("p3b", (C, 1), F32)
    ident = din("ident", (C, C))                     # identity for PE transpose

    p_out = nc.dram_tensor("p_out", [NB, 2, K], F32, kind="ExternalOutput").ap()
    h_out = nc.dram_tensor("h_out", [NB * K, RNN], F32, kind="ExternalOutput").ap()
    c_out = nc.dram_tensor("c_out", [NB * K, RNN], F32, kind="ExternalOutput").ap()

    with tile.TileContext(nc) as tc:
        with (
            tc.tile_pool(name="wp", bufs=1) as wp,
            tc.tile_pool(name="states", bufs=2) as statesp,
            tc.tile_pool(name="big", bufs=8) as bigp,
            tc.tile_pool(name="work", bufs=2) as work,
            tc.tile_pool(name="work3", bufs=3) as work3,
            tc.tile_pool(name="hp", bufs=2) as hp,
            tc.tile_pool(name="dram", bufs=3, space="DRAM") as dramp,
            tc.tile_pool(name="dram_states", bufs=NB, space="DRAM") as dram_states,
            tc.tile_pool(name="ps_st", bufs=1, space="PSUM") as ps_st,
            tc.tile_pool(name="ps_rt", bufs=2, space="PSUM") as ps_rt,
            tc.tile_pool(name="ps_sums", bufs=1, space="PSUM") as ps_sums,
            tc.tile_pool(name="ps_mm", bufs=3, space="PSUM") as ps_mm,
        ):
            # ---- load weights to SBUF once
            def wload(ap_in, shape, dt=BF16, tag=None):
                t = wp.tile(list(shape), dt, tag=tag or ap_in.tensor.name)
                nc.sync.dma_start(t[:], ap_in)
                return t

            wq_s = wload(wq, (C, NL, SD))
            wk_s = wload(wk, (C, NL, SD))
            wv_s = wload(wv, (C, NL, H * C))
            wfold_s = wload(wfold, (C, NL, 4, SD))
            wabot_s = wload(wabot, (C, NL, SD))
            cwt_s = wload(cwt, (C, NL, 9, SD))
            lng_s = wload(lng, (C, NL), F32)
            lnb_s = wload(lnb, (C, NL), F32)
            cbt_s = wload(cbt, (C, NL), F32)
            fusw_s = wload(fusw, (C, NL, FUS))
            fusb_s = wload(fusb, (C, 2), F32)
            rsgw_s = wload(rsgw, (C, 2, FUS))
            rssw_s = wload(rssw, (C, NL, FUS))
            rsb_s = wload(rsb, (C, 2), F32)
            wih_s = wload(wih, (C, 2, 4 * RNN))
            whh_s = wload(whh, (C, 2, 4 * RNN))
            p1ow_s = wload(p1ow, (C, 2, 256))
            p1gw_s = wload(p1gw, (C, 2, 256))
            p1sw_s = wload(p1sw, (C, NL, 256))
            p1b_s = wload(p1b, (C, 2), F32)
            p2w_s = wload(p2w, (C, 2, 64))
            p2b_s = wload(p2b, (C, 1), F32)
            p3w_s = wload(p3w, (C, 2))
            p3b_s = wload(p3b, (C, 1), F32)
            ident_s = wload(ident, (C, C))
            bcomb_bc = wp.tile([P, 4 * RNN], BF16, tag="bcomb_bc")
            nc.sync.dma_start(bcomb_bc[:], bcomb.to_broadcast((P, 4 * RNN)))
            onem = wp.tile([P, 32], BF16, tag="onem")   # value 1/128 (LN stats)
            nc.vector.memset(onem[:], 1.0 / 128.0)
            ones32 = wp.tile([P, 32], BF16, tag="ones32")
            nc.vector.memset(ones32[:], 1.0)
            eps_s = wp.tile([1, 1], F32, tag="eps_s")
            nc.vector.memset(eps_s[:], EPS)

            st_dram = [dram_states.tile([P, NL, K], BF16, tag="st_dram", name=f"st_dram_{b}") for b in range(NB)]

            # ================= main loop: 8 batches x 8 layers =================
            for b in range(NB):
                x0 = work.tile([P, K], BF16, tag="x0")
                nc.sync.dma_start(x0[:], xb[b])
                x_bf = x0
                for i in range(NL):
                    d = DIL[i]

                    # ---- LN stats (channels = partitions)
                    x_sq = work.tile([P, K], BF16, tag="x_sq")
                    nc.gpsimd.tensor_tensor(x_sq[:], x_bf, x_bf, op=ALU.mult)
                    mean_ps = ps_mm.tile([1, K], F32, tag="mm", name=f"mean_{b}_{i}")
                    nc.tensor.matmul(mean_ps[:], onem[:, 0:1], x_bf, start=True, stop=True)
                    ex2_ps = ps_mm.tile([1, K], F32, tag="mm", name=f"ex2_{b}_{i}")
                    nc.tensor.matmul(ex2_ps[:], onem[:, 0:1], x_sq[:], start=True, stop=True)
                    lnrow = work.tile([1, K], F32, tag="lnrow")
                    nc.scalar.activation(lnrow[:], mean_ps[:], func=AF.Square)
                    nc.vector.tensor_sub(lnrow[:], ex2_ps[:], lnrow[:])
                    nc.scalar.activation(lnrow[:], lnrow[:], func=AF.Ln, bias=eps_s[:])
                    ln_rows = work.tile([1, 2, K], BF16, tag="ln_rows")
                    # rstd = exp(-0.5*ln(var+eps))
                    nc.scalar.activation(ln_rows[:, 0, :], lnrow[:], func=AF.Exp, scale=-0.5)
                    # mu*rstd
                    nc.vector.tensor_mul(ln_rows[:, 1, :], mean_ps[:], ln_rows[:, 0, :])
                    ln_dram = dramp.tile([1, 2 * K], BF16, tag="ln_dram")
                    nc.sync.dma_start(ln_dram[:], ln_rows[:].rearrange("p a k -> p (a k)"))
                    ln_bc = work.tile([P, 2, K], BF16, tag="ln_bc")
                    nc.sync.dma_start(
                        ln_bc[:].rearrange("p a k -> p (a k)"),
                        ln_dram[:].to_broadcast((P, 2 * K)),
                    )
                    # ln = (x*rstd - mu*rstd)*g + b
                    t1 = work.tile([P, K], BF16, tag="t1")
                    nc.vector.tensor_mul(t1[:], x_bf, ln_bc[:, 0, :])
                    t2 = work.tile([P, K], BF16, tag="t2")
                    nc.vector.tensor_sub(t2[:], t1[:], ln_bc[:, 1, :])
                    ln_cf = work.tile([P, K], BF16, tag="ln_cf")
                    nc.vector.tensor_scalar(
                        ln_cf[:], t2[:], lng_s[:, i : i + 1], lnb_s[:, i : i + 1],
                        op0=ALU.mult, op1=ALU.add,
                    )

                    # ---- q^T, k^T  (feature-major, (128, 512))
                    qT_ps = ps_mm.tile([P, K], F32, tag="mm", name=f"q_{b}_{i}")
                    nc.tensor.matmul(qT_ps[:], wq_s[:, i, :], ln_cf[:], start=True, stop=True)
                    q_sb = work.tile([P, K], BF16, tag="q_sb")
                    nc.vector.tensor_copy(q_sb[:], qT_ps[:])
                    kT_ps = ps_mm.tile([P, K], F32, tag="mm", name=f"k_{b}_{i}")
                    nc.tensor.matmul(kT_ps[:], wk_s[:, i, :], ln_cf[:], start=True, stop=True)
                    k_sb = work.tile([P, K], BF16, tag="k_sb")
                    nc.vector.tensor_copy(k_sb[:], kT_ps[:])

                    # ---- v (points-major rows, (128 pts, 512 feat) per j-tile)
                    v_sb = work.tile([P, 4, H * C], BF16, tag="v_sb")
                    for jt in range(4):
                        v_ps = ps_mm.tile([P, H * C], F32, tag="mm", name=f"v_{b}_{i}_{jt}")
                        nc.tensor.matmul(
                            v_ps[:], ln_cf[:, jt * 128 : (jt + 1) * 128], wv_s[:, i, :],
                            start=True, stop=True,
                        )
                        if jt % 2 == 0:
                            nc.vector.tensor_copy(v_sb[:, jt, :], v_ps[:])
                        else:
                            nc.scalar.copy(v_sb[:, jt, :], v_ps[:])

                    # ---- S^T + exp, per j-tile; heads row-tiled, 2-head PSUM groups
                    at = [[None, None] for _ in range(4)]
                    for jt in range(4):
                        for g in range(2):
                            st_ps = ps_st.tile(
                                [P, 2, K], F32, tag="st", name=f"st_{b}_{i}_{jt}_{g}"
                            )
                            for hh in range(2):
                                h = 2 * g + hh
                                nc.tensor.matmul(
                                    st_ps[:, hh, :],
                                    k_sb[32 * h : 32 * (h + 1), jt * 128 : (jt + 1) * 128],
                                    q_sb[32 * h : 32 * (h + 1), :],
                                    start=True, stop=True, tile_position=(32 * h, 0),
                                )
                            a_t = bigp.tile(
                                [P, 2, K], BF16, tag="at", name=f"at_{b}_{i}_{jt}_{g}"
                            )
                            nc.scalar.activation(a_t[:], st_ps[:], func=AF.Exp)
                            at[jt][g] = a_t

                    # ---- softmax denominators: col-tiled ones-matmuls
                    sums_ps = ps_sums.tile([P, K], F32, tag="sums", name=f"sums_{b}_{i}")
                    for h in range(H):
                        for jt in range(4):
                            nc.tensor.matmul(
                                sums_ps[32 * h : 32 * (h + 1), :],
                                ones32[:], at[jt][h // 2][:, h % 2, :],
                                start=(jt == 0), stop=(jt == 3),
                                tile_position=(0, 32 * h),
                            )
                    rl = work.tile([P, K], F32, tag="rl")
                    nc.scalar.activation(rl[:], sums_ps[:], func=AF.Ln)
                    recip = work.tile([P, K], BF16, tag="recip")
                    nc.scalar.activation(recip[:], rl[:], func=AF.Exp, scale=-1.0)
                    rc_dram = dramp.tile([1, H * K], BF16, tag="rc_dram")
                    for h in range(H):
                        nc.sync.dma_start(
                            rc_dram[:, h * K : (h + 1) * K], recip[32 * h : 32 * h + 1, :]
                        )
                    rc_bc = work.tile([P, H, K], BF16, tag="rc_bc")
                    nc.sync.dma_start(
                        rc_bc[:].rearrange("p h k -> p (h k)"),
                        rc_dram[:].to_broadcast((P, H * K)),
                    )

                    # ---- r^T = v^T @ A^T (unnormalized), then normalize per head
                    r_sb = work.tile([P, H, K], BF16, tag="r_sb")
                    for h in range(H):
                        rt_ps = ps_rt.tile([P, K], F32, tag="rt", name=f"rt_{b}_{i}_{h}")
                        for jt in range(4):
                            nc.tensor.matmul(
                                rt_ps[:],
                                v_sb[:, jt, h * 128 : (h + 1) * 128],
                                at[jt][h // 2][:, h % 2, :],
                                start=(jt == 0), stop=(jt == 3),
                            )
                        nc.vector.tensor_mul(r_sb[:, h, :], rt_ps[:], rc_bc[:, h, :])

                    # ---- y^T = Wfold.T @ r^T + Wa_bot.T @ x
                    y_ps = ps_mm.tile([P, K], F32, tag="mm", name=f"y_{b}_{i}")
                    for kt in range(4):
                        nc.tensor.matmul(
                            y_ps[:], wfold_s[:, i, kt, :], r_sb[:, kt, :],
                            start=(kt == 0), stop=False,
                        )
                    nc.tensor.matmul(y_ps[:], wabot_s[:, i, :], x_bf, start=False, stop=True)

                    # ---- circular pad + dilated conv (9 taps)
                    y_pad = work.tile([P, K + 32], BF16, tag="y_pad")
                    nc.vector.tensor_copy(y_pad[:, 4 * d : 4 * d + K], y_ps[:])
                    nc.vector.tensor_copy(y_pad[:, 0 : 4 * d], y_pad[:, K : K + 4 * d])
                    nc.vector.tensor_copy(
                        y_pad[:, 4 * d + K : 8 * d + K], y_pad[:, 4 * d : 8 * d]
                    )
                    c_ps = ps_mm.tile([P, K], F32, tag="mm", name=f"c_{b}_{i}")
                    for t in range(9):
                        nc.tensor.matmul(
                            c_ps[:], cwt_s[:, i, t, :], y_pad[:, t * d : t * d + K],
                            start=(t == 0), stop=(t == 8),
                        )
                    # relu(conv + cb) (+ residual)
                    if i == 0:
                        h_new = hp.tile([P, K], F32, tag="h")
                        nc.vector.tensor_scalar(
                            h_new[:], c_ps[:], cbt_s[:, i : i + 1], 0.0,
                            op0=ALU.add, op1=ALU.max,
                        )
                    else:
                        t_relu = work.tile([P, K], F32, tag="t_relu")
                        nc.vector.tensor_scalar(
                            t_relu[:], c_ps[:], cbt_s[:, i : i + 1], 0.0,
                            op0=ALU.add, op1=ALU.max,
                        )
                        h_prev = h_cur
                        h_new = hp.tile([P, K], F32, tag="h")
                        nc.vector.tensor_add(h_new[:], t_relu[:], h_prev[:])
                    h_cur = h_new
                    hbf = work3.tile([P, K], BF16, tag="hbf")
                    nc.vector.tensor_copy(hbf[:], h_cur[:])
                    nc.sync.dma_start(st_dram[b][:, i, :], hbf[:])
                    x_bf = hbf

            # ================= tail: fusion + LSTM + head, per batch =================
            for b in range(NB):
                st_b = statesp.tile([P, NL, K], BF16, tag="st_b")
                nc.sync.dma_start(st_b[:], st_dram[b][:])
                # h0 channels-first via DMA transpose
                h0cf = work.tile([P, 2, K], BF16, tag="h0cf")
                for ct in range(2):
                    nc.sync.dma_start_transpose(
                        h0cf[:, ct, :],
                        h0b[b * K : (b + 1) * K, ct * 128 : (ct + 1) * 128],
                    )

                # fus conv1x1 + max over K -> g
                g_col = work.tile([P, 2, 1], F32, tag="g_col")
                g_bf = work.tile([P, 2, 1], BF16, tag="g_bf")
                for mt in range(2):
                    f_ps = ps_mm.tile([P, K], F32, tag="mm", name=f"f_{b}_{mt}")
                    for kt in range(NL):
                        nc.tensor.matmul(
                            f_ps[:], fusw_s[:, kt, mt * 128 : (mt + 1) * 128],
                            st_b[:, kt, :], start=(kt == 0), stop=(kt == 7),
                        )
                    gm = work.tile([P, 1], F32, tag="gm")
                    nc.vector.tensor_reduce(gm[:], f_ps[:], axis=mybir.AxisListType.X, op=ALU.max)
                    nc.vector.tensor_scalar(
                        g_col[:, mt, :], gm[:], fusb_s[:, mt : mt + 1], None, op0=ALU.add
                    )
                    nc.vector.tensor_copy(g_bf[:, mt, :], g_col[:, mt, :])

                # g contributions to rs and p1 (rank-1, N=1 matmuls) + biases
                rs_bias = work.tile([P, 2, 1], F32, tag="rs_bias")
                p1_bias = work.tile([P, 2, 1], F32, tag="p1_bias")
                for mt in range(2):
                    rg_ps = ps_mm.tile([P, 1], F32, tag="mm", name=f"rg_{b}_{mt}")
                    for ct in range(2):
                        nc.tensor.matmul(
                            rg_ps[:], rsgw_s[:, ct, mt * 128 : (mt + 1) * 128],
                            g_bf[:, ct, :], start=(ct == 0), stop=(ct == 1),
                        )
                    nc.vector.tensor_scalar(
                        rs_bias[:, mt, :], rg_ps[:], rsb_s[:, mt : mt + 1], None, op0=ALU.add
                    )
                    pg_ps = ps_mm.tile([P, 1], F32, tag="mm", name=f"pg_{b}_{mt}")
                    for ct in range(2):
                        nc.tensor.matmul(
                            pg_ps[:], p1gw_s[:, ct, mt * 128 : (mt + 1) * 128],
                            g_bf[:, ct, :], start=(ct == 0), stop=(ct == 1),
                        )
                    nc.vector.tensor_scalar(
                        p1_bias[:, mt, :], pg_ps[:], p1b_s[:, mt : mt + 1], None, op0=ALU.add
                    )

                # rnn_in channels-first
                rnn_cf = work.tile([P, 2, K], BF16, tag="rnn_cf")
                for mt in range(2):
                    rn_ps = ps_mm.tile([P, K], F32, tag="mm", name=f"rn_{b}_{mt}")
                    for kt in range(NL):
                        nc.tensor.matmul(
                            rn_ps[:], rssw_s[:, kt, mt * 128 : (mt + 1) * 128],
                            st_b[:, kt, :], start=(kt == 0), stop=(kt == 7),
                        )
                    nc.vector.tensor_scalar(
                        rnn_cf[:, mt, :], rn_ps[:], rs_bias[:, mt, :], None, op0=ALU.add
                    )

                # LSTM cell, points-major per k-tile
                o_cf = work.tile([P, 2, K], BF16, tag="o_cf")
                for kt in range(4):
                    gacts = []
                    for n2 in range(2):
                        g_ps = ps_mm.tile([P, 512], F32, tag="mm", name=f"g_{b}_{kt}_{n2}")
                        first = True
                        for ct in range(2):
                            nc.tensor.matmul(
                                g_ps[:], rnn_cf[:, ct, kt * 128 : (kt + 1) * 128],
                                wih_s[:, ct, n2 * 512 : (n2 + 1) * 512],
                                start=first, stop=False,
                            )
                            first = False
                        for ct in range(2):
                            nc.tensor.matmul(
                                g_ps[:], h0cf[:, ct, kt * 128 : (kt + 1) * 128],
                                whh_s[:, ct, n2 * 512 : (n2 + 1) * 512],
                                start=False, stop=(ct == 1),
                            )
                        ga = work.tile([P, 512], F32, tag="gact")
                        nc.vector.tensor_add(ga[:], g_ps[:], bcomb_bc[:, n2 * 512 : (n2 + 1) * 512])
                        gacts.append(ga)
                    # nonlinearities: [i,f] sigmoid, [g] tanh, [o] sigmoid
                    nc.scalar.activation(gacts[0][:], gacts[0][:], func=AF.Sigmoid)
                    nc.scalar.activation(gacts[1][:, 0:256], gacts[1][:, 0:256], func=AF.Tanh)
                    nc.scalar.activation(gacts[1][:, 256:512], gacts[1][:, 256:512], func=AF.Sigmoid)

                    c0t = work.tile([P, RNN], F32, tag="c0t")
                    nc.sync.dma_start(c0t[:], c0f[b * K + kt * 128 : b * K + (kt + 1) * 128, :])
                    t_a = work.tile([P, RNN], F32, tag="t_a")
                    nc.vector.tensor_mul(t_a[:], gacts[0][:, 256:512], c0t[:])
                    t_b = work.tile([P, RNN], F32, tag="t_b")
                    nc.gpsimd.tensor_tensor(
                        t_b[:], gacts[0][:, 0:256], gacts[1][:, 0:256], op=ALU.mult
                    )
                    c_new = work.tile([P, RNN], F32, tag="c_new")
                    nc.vector.tensor_add(c_new[:], t_a[:], t_b[:])
                    nc.sync.dma_start(
                        c_out[b * K + kt * 128 : b * K + (kt + 1) * 128, :], c_new[:]
                    )
                    tanh_c = work.tile([P, RNN], F32, tag="tanh_c")
                    nc.scalar.activation(tanh_c[:], c_new[:], func=AF.Tanh)
                    h_new = work.tile([P, RNN], F32, tag="hn")
                    nc.vector.tensor_mul(h_new[:], gacts[1][:, 256:512], tanh_c[:])
                    nc.sync.dma_start(
                        h_out[b * K + kt * 128 : b * K + (kt + 1) * 128, :], h_new[:]
                    )
                    hn_bf = work.tile([P, RNN], BF16, tag="hn_bf")
                    nc.vector.tensor_copy(hn_bf[:], h_new[:])
                    for ct in range(2):
                        tp_ps = ps_mm.tile([P, C], BF16, tag="mm", name=f"tp_{b}_{kt}_{ct}")
                        nc.tensor.transpose(
                            tp_ps[:], hn_bf[:, ct * 128 : (ct + 1) * 128], ident_s[:]
                        )
                        nc.vector.tensor_copy(o_cf[:, ct, kt * 128 : (kt + 1) * 128], tp_ps[:])

                # p1 -> p2 -> p3
                p1_sb = work.tile([P, 2, K], BF16, tag="p1_sb")
                for mt in range(2):
                    q_ps = ps_mm.tile([P, K], F32, tag="mm", name=f"p1_{b}_{mt}")
                    first = True
                    for ct in range(2):
                        nc.tensor.matmul(
                            q_ps[:], p1ow_s[:, ct, mt * 128 : (mt + 1) * 128],
                            o_cf[:, ct, :], start=first, stop=False,
                        )
                        first = False
                    for kt in range(NL):
                        nc.tensor.matmul(
                            q_ps[:], p1sw_s[:, kt, mt * 128 : (mt + 1) * 128],
                            st_b[:, kt, :], start=False, stop=(kt == 7),
                        )
                    nc.vector.tensor_scalar(
                        p1_sb[:, mt, :], q_ps[:], p1_bias[:, mt, :], 0.0,
                        op0=ALU.add, op1=ALU.max,
                    )
                p2_ps = ps_mm.tile([P, K], F32, tag="mm", name=f"p2_{b}")
                for ct in range(2):
                    nc.tensor.matmul(
                        p2_ps[0:64, :], p2w_s[:, ct, :], p1_sb[:, ct, :],
                        start=(ct == 0), stop=(ct == 1),
                    )
                p2_sb = work.tile([64, K], BF16, tag="p2_sb")
                nc.vector.tensor_scalar(
                    p2_sb[:], p2_ps[0:64, :], p2b_s[0:64, :], 0.0, op0=ALU.add, op1=ALU.max
                )
                p3_ps = ps_mm.tile([P, K], F32, tag="mm", name=f"p3_{b}")
                nc.tensor.matmul(p3_ps[0:2, :], p3w_s[0:64, :], p2_sb[:], start=True, stop=True)
                p_sb = work.tile([2, K], F32, tag="p_sb")
                nc.vector.tensor_scalar(
                    p_sb[:], p3_ps[0:2, :], p3b_s[0:2, :], None, op0=ALU.add
                )
                nc.sync.dma_start(p_out[b], p_sb[:])

    nc.compile()
    return nc


def _prep_inputs(inputs):
    """Host-side weight prep (shared across cores)."""
    f32 = np.float32
    Wq = np.asarray(inputs["Wq"], f32)
    Wk = np.asarray(inputs["Wk"], f32)
    Wv = np.asarray(inputs["Wv"], f32)
    Ww = np.asarray(inputs["Ww"], f32)
    Wa = np.asarray(inputs["Wa"], f32)
    cw = np.asarray(inputs["cw"], f32)

    scale = 1.0 / np.sqrt(np.float32(SD // 4))
    wq = np.ascontiguousarray((Wq * scale).transpose(1, 0, 2)).astype(bf16)  # (C, NL, SD)
    wk = np.ascontiguousarray(Wk.transpose(1, 0, 2)).astype(bf16)
    wv = np.ascontiguousarray(Wv.transpose(1, 0, 2)).astype(bf16)
    # fold Ww @ Wa_top : (NL, 512, 128)
    wf = np.einsum("lfk,lko->lfo", Ww.astype(np.float64), Wa[:, :SD, :].astype(np.float64)).astype(f32)
    wfold = np.ascontiguousarray(
        wf.reshape(NL, 4, SD, SD).transpose(2, 0, 1, 3)
    ).astype(bf16)                                                           # (C, NL, 4, SD)
    wabot = np.ascontiguousarray(Wa[:, SD:, :].transpose(1, 0, 2)).astype(bf16)
    cwt = np.ascontiguousarray(cw.transpose(2, 0, 3, 1)).astype(bf16)        # (C, NL, 9, SD)

    fus_w = np.asarray(inputs["fus_w"], f32)       # (256, 1024)
    fusw = np.ascontiguousarray(
        fus_w.T.reshape(NL, 128, FUS).transpose(1, 0, 2)
    ).astype(bf16)                                                           # (C, NL, FUS)
    rs_w = np.asarray(inputs["rs_w"], f32)         # (256, 1280)
    rsgw = np.ascontiguousarray(
        rs_w[:, :FUS].T.reshape(2, 128, FUS).transpose(1, 0, 2)
    ).astype(bf16)
    rssw = np.ascontiguousarray(
        rs_w[:, FUS:].T.reshape(NL, 128, FUS).transpose(1, 0, 2)
    ).astype(bf16)
    W_ih = np.asarray(inputs["W_ih"], f32)         # (1024, 256)
    W_hh = np.asarray(inputs["W_hh"], f32)
    wih = np.ascontiguousarray(W_ih.T.reshape(2, 128, 4 * RNN)).transpose(1, 0, 2)
    wih = np.ascontiguousarray(wih).astype(bf16)
    whh = np.ascontiguousarray(W_hh.T.reshape(2, 128, 4 * RNN).transpose(1, 0, 2)).astype(bf16)
    bcomb = ((np.asarray(inputs["b_ih"], f32) + np.asarray(inputs["b_hh"], f32)).reshape(1, -1)).astype(bf16)

    p1_w = np.asarray(inputs["p1_w"], f32)         # (256, 1536)
    p1ow = np.ascontiguousarray(p1_w[:, :256].T.reshape(2, 128, 256).transpose(1, 0, 2)).astype(bf16)
    p1gw = np.ascontiguousarray(p1_w[:, 256:512].T.reshape(2, 128, 256).transpose(1, 0, 2)).astype(bf16)
    p1sw = np.ascontiguousarray(p1_w[:, 512:].T.reshape(NL, 128, 256).transpose(1, 0, 2)).astype(bf16)
    p2_w = np.asarray(inputs["p2_w"], f32)         # (64, 256)
    p2w = np.ascontiguousarray(p2_w.T.reshape(2, 128, 64)).transpose(1, 0, 2)
    p2w = np.ascontiguousarray(p2w).astype(bf16)
    p3_w = np.asarray(inputs["p3_w"], f32)         # (2, 64)
    p3w = np.zeros((C, 2), f32)
    p3w[:64, :] = p3_w.T
    p3w = p3w.astype(bf16)

    def cols(v, n):
        return np.ascontiguousarray(np.asarray(v, f32).reshape(n, C).T)

    w = dict(
        wq=wq, wk=wk, wv=wv, wfold=wfold, wabot=wabot, cwt=cwt,
        lng=cols(inputs["ln_g"], NL), lnb=cols(inputs["ln_b"], NL),
        cbt=cols(inputs["cb"], NL),
        fusw=fusw, fusb=cols(inputs["fus_b"], 2),
        rsgw=rsgw, rssw=rssw, rsb=cols(inputs["rs_b"], 2),
        wih=wih, whh=whh, bcomb=bcomb,
        p1ow=p1ow, p1gw=p1gw, p1sw=p1sw, p1b=cols(inputs["p1_b"], 2),
        p2w=p2w, p2b=np.concatenate([np.asarray(inputs["p2_b"], f32), np.zeros(64, f32)]).reshape(C, 1),
        p3w=p3w, p3b=np.concatenate([np.asarray(inputs["p3_b"], f32), np.zeros(126, f32)]).reshape(C, 1),
        ident=np.eye(C, dtype=f32).astype(bf16),
    )
    return w


def kernel(**inputs):
    if "nc" not in _CACHE:
        _CACHE["nc"] = _build()
    nc = _CACHE["nc"]

    w = _prep_inputs(inputs)
    x = np.asarray(inputs["x"], np.float32)
    h0 = np.asarray(inputs["h0"], np.float32)[0]
    c0 = np.asarray(inputs["c0"], np.float32)[0]

    in_maps = []
    for c in range(NCORES):
        bs = c * NB
        m = dict(w)
        m["xb"] = np.ascontiguousarray(x[bs : bs + NB]).astype(bf16)
        m["h0b"] = np.ascontiguousarray(h0[bs * K : (bs + NB) * K]).astype(bf16)
        m["c0f"] = np.ascontiguousarray(c0[bs * K : (bs + NB) * K])
        in_maps.append(m)

    res = run_bass_kernel_spmd(nc, in_maps, core_ids=list(range(NCORES)))

    p = np.concatenate([res.results[c]["p_out"] for c in range(NCORES)], axis=0)
    h_new = np.concatenate([res.results[c]["h_out"] for c in range(NCORES)], axis=0)
    c_new = np.concatenate([res.results[c]["c_out"] for c in range(NCORES)], axis=0)
    return (
        p.astype(np.float32),
        h_new[None].astype(np.float32),
        c_new[None].astype(np.float32),
    )
